# revision 18
# baseline (speedup 1.0000x reference)
"""Trainium2 Bass kernel for nn_BasicModel_47871705481510.

Math: per qubit i, with v_i = w_i + (x[0] if i==0 else x[1] if i==1 else 0):
  state_i = [cos(v_i/2), -i*sin(v_i/2)]^T   (Rx(w) @ Rx(theta1) |0> collapses
                                             to a single rotation by v_i)
  O_i     = cos(v_i)                         (real)

Device computes three f32 planes per qubit via the ACT Sin table (accurate
only for |arg| <= pi, hence the quarter/half-angle forms):
  sneg = sin(-v/2)            (= imag part of state[1])
  c    = 1 - 2*sin^2(v/4)     (= cos(v/2), real part of state[0])
  cosv = 1 - 2*sin^2(v/2)     (= cos(v) = O)
Host assembles the complex64 outputs (structural zeros / interleave only).

Sharding: data-parallel over qubits. 2,000,000 qubits padded to 8 * 128 * 2048
= 2,097,152; each of the 8 cores handles one contiguous [128, 2048] f32 shard.
x is passed as a [1,2] tensor: the real x to core 0 (whose shard holds qubits
0,1 at partition 0, cols 0,1), zeros to the other cores, so the SPMD program
is uniform.
"""

import numpy as np

N = 2_000_000
N_CORES = 8
P = 128
F = 2048
PER_CORE = P * F  # 262144
N_PAD = PER_CORE * N_CORES

# compute chunks (columns): small first chunk for fast pipeline ramp
CHS = [256, 768, 512, 512]
SQ_ON_ACT = {0, 1}  # chunks whose square runs on ACT; rest on DVE

_cache = {}


def _build_nc_raw():
    """Raw Bacc kernel (no TileContext): hand-placed semaphores, distinct
    SBUF buffers (no reuse -> only RAW deps need sems), no Tile tail
    machinery. Streams:
      Scalar: warm-up Sin (hoists ACT_TABLE_LOAD), even in-chunks, ACT ops,
              o-plane half-outs (scalar HWDGE queue)
      Sync:   odd in-chunks, c-plane half-outs (sync HWDGE queue)
      Pool:   x DMA + s-plane half-outs (SWDGE queue)
      Vector: x add, u^2 muls, some squares, the two affines per chunk
    Outputs are written as two [128,1024] DMAs per plane (4KB rows ->
    fewer descriptors than per-chunk 2KB rows).
    """
    import concourse.bacc as bacc
    import concourse.mybir as mybir

    nc = bacc.Bacc(
        "TRN2", target_bir_lowering=False, debug=False, num_devices=N_CORES
    )
    w_in = nc.declare_dram_parameter("w", [P, F], mybir.dt.float32, isOutput=False)
    x_in = nc.declare_dram_parameter("x", [1, 2], mybir.dt.float32, isOutput=False)
    c_out = nc.declare_dram_parameter("c", [P, F], mybir.dt.float32, isOutput=True)
    s_out = nc.declare_dram_parameter("s", [P, F], mybir.dt.float32, isOutput=True)
    o_out = nc.declare_dram_parameter("o", [P, F], mybir.dt.float32, isOutput=True)

    SIN = mybir.ActivationFunctionType.Sin
    SQUARE = mybir.ActivationFunctionType.Square
    MULT = mybir.AluOpType.mult
    ADD = mybir.AluOpType.add
    f32 = mybir.dt.float32

    NCH = len(CHS)
    offs = [sum(CHS[:j]) for j in range(NCH)]
    assert sum(CHS) == F
    HALF = F // 2
    # chunk -> (half index, column range within half)
    half_of = []
    for j in range(NCH):
        h = 0 if offs[j] < HALF else 1
        assert offs[j] + CHS[j] <= HALF or offs[j] >= HALF, "chunk crosses half"
        half_of.append((h, slice(offs[j] - h * HALF, offs[j] + CHS[j] - h * HALF)))
    last_of_half = [max(j for j in range(NCH) if half_of[j][0] == h) for h in (0, 1)]

    wt = [nc.alloc_sbuf_tensor(f"wt{j}", [P, CHS[j]], f32).ap() for j in range(NCH)]
    ut = [nc.alloc_sbuf_tensor(f"ut{j}", [P, CHS[j]], f32).ap() for j in range(NCH)]
    u2 = [nc.alloc_sbuf_tensor(f"u2{j}", [P, CHS[j]], f32).ap() for j in range(NCH)]
    s2 = [nc.alloc_sbuf_tensor(f"s2{j}", [P, CHS[j]], f32).ap() for j in range(NCH)]
    stH = [nc.alloc_sbuf_tensor(f"st{h}", [P, HALF], f32).ap() for h in (0, 1)]
    ctH = [nc.alloc_sbuf_tensor(f"ct{h}", [P, HALF], f32).ap() for h in (0, 1)]
    otH = [nc.alloc_sbuf_tensor(f"ot{h}", [P, HALF], f32).ap() for h in (0, 1)]
    xt = nc.alloc_sbuf_tensor("xt", [1, 2], f32).ap()
    warm = nc.alloc_sbuf_tensor("warm", [1, 1], f32).ap()
    zconst = nc.const_aps.tensor(0.0, (1, 1), f32)

    def st_ap(j):
        h, r = half_of[j]
        return stH[h][:, r]

    def ct_ap(j):
        h, r = half_of[j]
        return ctH[h][:, r]

    def ot_ap(j):
        h, r = half_of[j]
        return otH[h][:, r]

    in_sl = [slice(offs[j], offs[j] + CHS[j]) for j in range(NCH)]

    # ---- dry pass: semaphore counter schedule ----
    au = [0] * NCH   # act count after sin_u[j]
    as_ = [0] * NCH  # after sin_s[j]
    asq = {}         # after ACT square[j]
    a = 0
    for j in range(NCH):
        a += 1
        au[j] = a
        a += 1
        as_[j] = a
        if j in SQ_ON_ACT:
            a += 1
            asq[j] = a
    dmu = [0] * NCH  # dve count after mul_u2[j]
    dct = [0] * NCH  # after aff_ct[j]
    ds2 = {}         # after DVE mul_s2[j]
    dot = [0] * NCH  # after aff_ot[j]
    d = 0
    for j in range(NCH):
        d += 1
        dmu[j] = d
        d += 1
        dct[j] = d
        if j not in SQ_ON_ACT:
            d += 1
            ds2[j] = d
        d += 1
        dot[j] = d

    from contextlib import ExitStack

    with ExitStack() as stack:
        in_sem = [stack.enter_context(nc.semaphore(f"in{j}")) for j in range(NCH)]
        xt_sem = stack.enter_context(nc.semaphore("xt_sem"))
        act_sem = stack.enter_context(nc.semaphore("act_sem"))
        add_sem = stack.enter_context(nc.semaphore("add_sem"))
        dve_sem = stack.enter_context(nc.semaphore("dve_sem"))
        out_sc = stack.enter_context(nc.semaphore("out_sc"))
        out_sy = stack.enter_context(nc.semaphore("out_sy"))
        out_po = stack.enter_context(nc.semaphore("out_po"))
        block = stack.enter_context(nc.Block())

        @block.scalar
        def _(scalar):
            scalar.activation(warm, zconst, SIN)  # pulls ACT_TABLE_LOAD early
            for j in range(0, NCH, 2):
                scalar.dma_start(wt[j], w_in[:, in_sl[j]]).then_inc(in_sem[j], 16)
            outs_done = 0
            for j in range(NCH):
                scalar.wait_ge(in_sem[j], 16)
                if j == 0:
                    scalar.wait_ge(add_sem, 1)
                scalar.activation(ut[j], wt[j], SIN, scale=0.25).then_inc(act_sem, 1)
                scalar.activation(st_ap(j), wt[j], SIN, scale=-0.5).then_inc(
                    act_sem, 1
                )
                if j in SQ_ON_ACT:
                    # same-engine RAW (st) across the deep ACT pipeline
                    scalar.wait_ge(act_sem, as_[j])
                    scalar.activation(s2[j], st_ap(j), SQUARE).then_inc(act_sem, 1)
                for h in (0, 1):
                    # o-plane half h out as soon as its last chunk's ot is done
                    if j == last_of_half[h] and j != NCH - 1:
                        scalar.wait_ge(dve_sem, dot[j])
                        scalar.dma_start(
                            o_out[:, h * HALF : (h + 1) * HALF], otH[h]
                        ).then_inc(out_sc, 16)
                        outs_done += 1
            for h in (0, 1):
                if last_of_half[h] == NCH - 1:
                    scalar.wait_ge(dve_sem, dot[NCH - 1])
                    scalar.dma_start(
                        o_out[:, h * HALF : (h + 1) * HALF], otH[h]
                    ).then_inc(out_sc, 16)
                    outs_done += 1
            assert outs_done == 2
            scalar.wait_ge(out_sc, 32)

        @block.sync
        def _(sync):
            for j in range(1, NCH, 2):
                sync.dma_start(wt[j], w_in[:, in_sl[j]]).then_inc(in_sem[j], 16)
            for h in (0, 1):
                sync.wait_ge(dve_sem, dct[last_of_half[h]])
                sync.dma_start(
                    c_out[:, h * HALF : (h + 1) * HALF], ctH[h]
                ).then_inc(out_sy, 16)
            sync.wait_ge(out_sy, 32)

        @block.gpsimd
        def _(gpsimd):
            gpsimd.dma_start(xt, x_in[:]).then_inc(xt_sem, 16)
            for h in (0, 1):
                gpsimd.wait_ge(act_sem, as_[last_of_half[h]])
                gpsimd.dma_start(
                    s_out[:, h * HALF : (h + 1) * HALF], stH[h]
                ).then_inc(out_po, 16)
            gpsimd.wait_ge(out_po, 32)

        @block.vector
        def _(vector):
            vector.wait_ge(xt_sem, 16)
            vector.wait_ge(in_sem[0], 16)
            vector.tensor_add(
                wt[0][0:1, 0:2], wt[0][0:1, 0:2], xt[0:1, 0:2]
            ).then_inc(add_sem, 1)
            for j in range(NCH):
                vector.wait_ge(act_sem, au[j])
                vector.tensor_mul(u2[j], ut[j], ut[j]).then_inc(dve_sem, 1)
                # same-engine RAW (u2) across the deep DVE pipeline
                vector.wait_ge(dve_sem, dmu[j])
                vector.tensor_scalar(
                    ct_ap(j), u2[j], -2.0, 1.0, MULT, ADD
                ).then_inc(dve_sem, 1)
                if j in SQ_ON_ACT:
                    vector.wait_ge(act_sem, asq[j])
                else:
                    vector.wait_ge(act_sem, as_[j])
                    vector.tensor_mul(s2[j], st_ap(j), st_ap(j)).then_inc(dve_sem, 1)
                    vector.wait_ge(dve_sem, ds2[j])
                vector.tensor_scalar(
                    ot_ap(j), s2[j], -2.0, 1.0, MULT, ADD
                ).then_inc(dve_sem, 1)

    nc.finalize()
    return nc


USE_RAW = True


def _get_nc():
    if "nc" not in _cache:
        _cache["nc"] = _build_nc_raw()
    return _cache["nc"]


def _run(x, w, **spmd_kwargs):
    """Shard, run on 8 cores, return (c, sneg, cosv) full f32 vectors plus
    the raw BassKernelResults (for profiling from test harnesses)."""
    from concourse.bass_utils import run_bass_kernel_spmd

    x = np.ascontiguousarray(np.asarray(x, dtype=np.float32)).reshape(1, 2)
    w = np.asarray(w, dtype=np.float32).reshape(-1)
    assert w.shape[0] == N
    w_pad = np.zeros(N_PAD, dtype=np.float32)
    w_pad[:N] = w
    shards = w_pad.reshape(N_CORES, P, F)
    zero_x = np.zeros((1, 2), dtype=np.float32)
    in_maps = [
        {"w": shards[i], "x": (x if i == 0 else zero_x)} for i in range(N_CORES)
    ]
    res = run_bass_kernel_spmd(_get_nc(), in_maps, list(range(N_CORES)), **spmd_kwargs)
    c = np.concatenate([r["c"].reshape(-1) for r in res.results])[:N]
    sneg = np.concatenate([r["s"].reshape(-1) for r in res.results])[:N]
    cosv = np.concatenate([r["o"].reshape(-1) for r in res.results])[:N]
    return c, sneg, cosv, res


def kernel(x, w):
    c, sneg, cosv, _ = _run(x, w)
    state = np.zeros((N, 4), dtype=np.float32)
    state[:, 0] = c
    state[:, 3] = sneg
    state = state.view(np.complex64).reshape(N, 2, 1)
    O = np.zeros((N, 2), dtype=np.float32)
    O[:, 0] = cosv
    O = O.view(np.complex64).reshape(N, 1, 1)
    return state, O


# revision 23
# speedup vs baseline: 1.0330x; 1.0330x over previous
"""Trainium2 Bass kernel for nn_BasicModel_47871705481510.

Math: per qubit i, with v_i = w_i + (x[0] if i==0 else x[1] if i==1 else 0):
  state_i = [cos(v_i/2), -i*sin(v_i/2)]^T   (Rx(w) @ Rx(theta1) |0> collapses
                                             to a single rotation by v_i)
  O_i     = cos(v_i)                         (real)

Device computes three f32 planes per qubit. The ACT Sin table is accurate
only for |arg| <= pi (max |v| here is ~5.42), so:
  sneg = sin(-v/2)                 |v/2| <= 2.71 < pi         (ACT)
  c    = sin(pi/2 - |v|/2)         arg in [-1.14, pi/2]       (DVE abs + ACT)
  cosv = 1 - 2*sneg^2                                          (DVE)
Host assembles the complex64 outputs (structural zeros / interleave only).

Sharding: data-parallel over qubits. 2,000,000 qubits padded to 8 * 128 * 2048
= 2,097,152; each of the 8 cores handles one contiguous [128, 2048] f32 shard.
x is passed as a [1,2] tensor: the real x to core 0 (whose shard holds qubits
0,1 at partition 0, cols 0,1), zeros to the other cores, so the SPMD program
is uniform. A [128,1] pi/2 bias column is shipped as a tiny constant input.
"""

import numpy as np

N = 2_000_000
N_CORES = 8
P = 128
F = 2048
PER_CORE = P * F  # 262144
N_PAD = PER_CORE * N_CORES

# compute chunks (columns): small first chunk for fast pipeline ramp
CHS = [256, 768, 512, 512]
HALF = F // 2

_cache = {}


def _build_nc_raw():
    """Raw Bacc kernel (no TileContext): hand-placed semaphores, distinct
    SBUF buffers (no reuse -> only RAW deps need sems), no Tile tail
    machinery. Streams:
      Sync:   in0/in1 (this HWDGE queue wakes ~1.4us earlier than scalar's),
              o-plane outs
      Scalar: warm-up Sin (hoists ACT_TABLE_LOAD), in2/in3, all ACT ops,
              c-plane outs
      Pool:   x + pi/2-bias DMAs, s-plane outs (SWDGE queue)
      Vector: x add; per chunk: |v|/2, sneg^2, cosv affine
    Outputs: cols 0-1023 as one [128,1024] DMA per plane, then two [128,512]
    DMAs per plane (finer grain shortens the drain after the last compute).
    """
    import concourse.bacc as bacc
    import concourse.mybir as mybir

    nc = bacc.Bacc(
        "TRN2", target_bir_lowering=False, debug=False, num_devices=N_CORES
    )
    w_in = nc.declare_dram_parameter("w", [P, F], mybir.dt.float32, isOutput=False)
    x_in = nc.declare_dram_parameter("x", [1, 2], mybir.dt.float32, isOutput=False)
    b_in = nc.declare_dram_parameter("b", [P, 1], mybir.dt.float32, isOutput=False)
    c_out = nc.declare_dram_parameter("c", [P, F], mybir.dt.float32, isOutput=True)
    s_out = nc.declare_dram_parameter("s", [P, F], mybir.dt.float32, isOutput=True)
    o_out = nc.declare_dram_parameter("o", [P, F], mybir.dt.float32, isOutput=True)

    SIN = mybir.ActivationFunctionType.Sin
    MULT = mybir.AluOpType.mult
    ADD = mybir.AluOpType.add
    AND = mybir.AluOpType.bitwise_and
    f32 = mybir.dt.float32
    i32 = mybir.dt.int32

    NCH = len(CHS)
    offs = [sum(CHS[:j]) for j in range(NCH)]
    assert sum(CHS) == F

    def half_slice(j):
        h = 0 if offs[j] < HALF else 1
        assert offs[j] + CHS[j] <= HALF or offs[j] >= HALF
        return h, slice(offs[j] - h * HALF, offs[j] + CHS[j] - h * HALF)

    wt = [nc.alloc_sbuf_tensor(f"wt{j}", [P, CHS[j]], f32).ap() for j in range(NCH)]
    at = [nc.alloc_sbuf_tensor(f"at{j}", [P, CHS[j]], f32).ap() for j in range(NCH)]
    s2 = [nc.alloc_sbuf_tensor(f"s2{j}", [P, CHS[j]], f32).ap() for j in range(NCH)]
    stH = [nc.alloc_sbuf_tensor(f"st{h}", [P, HALF], f32).ap() for h in (0, 1)]
    ctH = [nc.alloc_sbuf_tensor(f"ct{h}", [P, HALF], f32).ap() for h in (0, 1)]
    otH = [nc.alloc_sbuf_tensor(f"ot{h}", [P, HALF], f32).ap() for h in (0, 1)]
    xt = nc.alloc_sbuf_tensor("xt", [1, 2], f32).ap()
    bt = nc.alloc_sbuf_tensor("bt", [P, 1], f32).ap()
    warm = nc.alloc_sbuf_tensor("warm", [1, 1], f32).ap()
    zconst = nc.const_aps.tensor(0.0, (1, 1), f32)

    def st_ap(j):
        h, r = half_slice(j)
        return stH[h][:, r]

    def ct_ap(j):
        h, r = half_slice(j)
        return ctH[h][:, r]

    def ot_ap(j):
        h, r = half_slice(j)
        return otH[h][:, r]

    in_sl = [slice(offs[j], offs[j] + CHS[j]) for j in range(NCH)]

    # act counts: per chunk j: st_j -> 2j+1, c_j -> 2j+2
    # dve counts: per chunk j: a_j -> 3j+1, s2_j -> 3j+2, o_j -> 3j+3
    # output pieces: (full-plane col range, SBUF source fn, last chunk)
    pieces = [
        (slice(0, HALF), lambda T: T[0], 1),
        (slice(HALF, HALF + 512), lambda T: T[1][:, 0:512], 2),
        (slice(HALF + 512, F), lambda T: T[1][:, 512:1024], 3),
    ]

    from contextlib import ExitStack

    with ExitStack() as stack:
        in_sem = [stack.enter_context(nc.semaphore(f"in{j}")) for j in range(NCH)]
        xt_sem = stack.enter_context(nc.semaphore("xt_sem"))
        bt_sem = stack.enter_context(nc.semaphore("bt_sem"))
        act_sem = stack.enter_context(nc.semaphore("act_sem"))
        add_sem = stack.enter_context(nc.semaphore("add_sem"))
        dve_sem = stack.enter_context(nc.semaphore("dve_sem"))
        out_sc = stack.enter_context(nc.semaphore("out_sc"))
        out_sy = stack.enter_context(nc.semaphore("out_sy"))
        out_po = stack.enter_context(nc.semaphore("out_po"))
        block = stack.enter_context(nc.Block())

        @block.sync
        def _(sync):
            for j in (0, 1):
                sync.dma_start(wt[j], w_in[:, in_sl[j]]).then_inc(in_sem[j], 16)
            for pj, (colr, src, lastj) in enumerate(pieces):
                sync.wait_ge(dve_sem, 3 * lastj + 3)
                sync.dma_start(o_out[:, colr], src(otH)).then_inc(out_sy, 16)
            sync.wait_ge(out_sy, 48)

        @block.scalar
        def _(scalar):
            scalar.activation(warm, zconst, SIN)  # pulls ACT_TABLE_LOAD early
            for j in (2, 3):
                scalar.dma_start(wt[j], w_in[:, in_sl[j]]).then_inc(in_sem[j], 16)
            pidx = 0
            for j in range(NCH):
                scalar.wait_ge(in_sem[j], 16)
                if j == 0:
                    scalar.wait_ge(add_sem, 1)
                scalar.activation(st_ap(j), wt[j], SIN, scale=-0.5).then_inc(
                    act_sem, 1
                )
                if j == 0:
                    scalar.wait_ge(bt_sem, 16)
                scalar.wait_ge(dve_sem, 3 * j + 1)  # a_j ready
                scalar.activation(ct_ap(j), at[j], SIN, bias=bt, scale=-0.5).then_inc(
                    act_sem, 1
                )
                # c-plane pieces go out as soon as their last chunk's c is done
                while pidx < len(pieces) and pieces[pidx][2] == j:
                    colr, src, _lastj = pieces[pidx]
                    scalar.wait_ge(act_sem, 2 * j + 2)
                    scalar.dma_start(c_out[:, colr], src(ctH)).then_inc(out_sc, 16)
                    pidx += 1
            assert pidx == len(pieces)
            scalar.wait_ge(out_sc, 48)

        @block.gpsimd
        def _(gpsimd):
            gpsimd.dma_start(xt, x_in[:]).then_inc(xt_sem, 16)
            gpsimd.dma_start(bt, b_in[:]).then_inc(bt_sem, 16)
            for colr, src, lastj in pieces:
                gpsimd.wait_ge(act_sem, 2 * lastj + 1)
                gpsimd.dma_start(s_out[:, colr], src(stH)).then_inc(out_po, 16)
            gpsimd.wait_ge(out_po, 48)

        @block.vector
        def _(vector):
            vector.wait_ge(xt_sem, 16)
            vector.wait_ge(in_sem[0], 16)
            vector.tensor_add(
                wt[0][0:1, 0:2], wt[0][0:1, 0:2], xt[0:1, 0:2]
            ).then_inc(add_sem, 1)
            for j in range(NCH):
                if j > 0:
                    vector.wait_ge(in_sem[j], 16)
                else:
                    # same-engine RAW: the x add wrote wt0 through the DVE pipe
                    vector.wait_ge(add_sem, 1)
                # a_j = |v| via sign-bit clear on the int32 view
                vector.tensor_scalar(
                    at[j].bitcast(i32), wt[j].bitcast(i32), 0x7FFFFFFF, None, AND
                ).then_inc(dve_sem, 1)
                vector.wait_ge(act_sem, 2 * j + 1)  # st_j ready
                vector.tensor_mul(s2[j], st_ap(j), st_ap(j)).then_inc(dve_sem, 1)
                vector.wait_ge(dve_sem, 3 * j + 2)  # same-engine RAW (s2)
                vector.tensor_scalar(
                    ot_ap(j), s2[j], -2.0, 1.0, MULT, ADD
                ).then_inc(dve_sem, 1)

    nc.finalize()
    return nc


def _get_nc():
    if "nc" not in _cache:
        _cache["nc"] = _build_nc_raw()
    return _cache["nc"]


def _run(x, w, **spmd_kwargs):
    """Shard, run on 8 cores, return (c, sneg, cosv) full f32 vectors plus
    the raw BassKernelResults (for profiling from test harnesses)."""
    from concourse.bass_utils import run_bass_kernel_spmd

    x = np.ascontiguousarray(np.asarray(x, dtype=np.float32)).reshape(1, 2)
    w = np.asarray(w, dtype=np.float32).reshape(-1)
    assert w.shape[0] == N
    w_pad = np.zeros(N_PAD, dtype=np.float32)
    w_pad[:N] = w
    shards = w_pad.reshape(N_CORES, P, F)
    zero_x = np.zeros((1, 2), dtype=np.float32)
    bias = np.full((P, 1), np.float32(np.pi / 2), dtype=np.float32)
    in_maps = [
        {"w": shards[i], "x": (x if i == 0 else zero_x), "b": bias}
        for i in range(N_CORES)
    ]
    res = run_bass_kernel_spmd(_get_nc(), in_maps, list(range(N_CORES)), **spmd_kwargs)
    c = np.concatenate([r["c"].reshape(-1) for r in res.results])[:N]
    sneg = np.concatenate([r["s"].reshape(-1) for r in res.results])[:N]
    cosv = np.concatenate([r["o"].reshape(-1) for r in res.results])[:N]
    return c, sneg, cosv, res


def kernel(x, w):
    c, sneg, cosv, _ = _run(x, w)
    state = np.zeros((N, 4), dtype=np.float32)
    state[:, 0] = c
    state[:, 3] = sneg
    state = state.view(np.complex64).reshape(N, 2, 1)
    O = np.zeros((N, 2), dtype=np.float32)
    O[:, 0] = cosv
    O = O.view(np.complex64).reshape(N, 1, 1)
    return state, O
